# revision 6
# baseline (speedup 1.0000x reference)
"""FM layer (factorization machine) Trainium2 Bass kernel.

Computes, for x (B=16384, N=4096), W (1, N), b (1,), V (N, K=128):
    out = x @ W.T + b + 0.5*sum((x@V)**2, axis=1)
          - 0.5*||V.sum(0)||^2 * (x.sum(1))**2

Data-parallel over B across 8 NeuronCores (2048 rows/core). The kernel is
HBM-bound: reading x (33.5 MB/core fp32) at the ~2.9 TB/s chip HBM ceiling
takes ~93 us; all compute hides under the stream.

Per-core design (p-major rows: SBUF partition p holds rows [16p, 16p+16)):
- x streams HBM->SBUF fp32 via HWDGE on one queue in big tapered chunks
  (8 MB head reads -> one contiguous 64 KB+ run per partition; 2 MB tail
  chunks so late tiles arrive spread out; the final tile in two
  column-half DMAs so its cast starts before the last byte). 2-deep ring;
  small consts ([V|W.T|1] pre-shuffled fp16 + aux) FIFO right after
  chunk 0 on the same queue (a second HWDGE queue gets starved by
  packet-level round-robin against 64 KB descriptors; SWDGE descriptor
  generation is far too slow).
- Per 128-row m-tile: fp32->fp16 cast split ACT cols [0,3072) / DVE rest;
  PE identity-transposes the 32 n-chunks (fp16, ~66 ns each); DVE copies
  each full PSUM bank (8 transposes) back to SBUF; PE matmuls
  y = x_tile @ [V | W.T | 1] accumulating fp32 in PSUM; all-DVE epilogue
  (STT with accum_out gives sum(0.5*y^2) in one op).
- Tail balancing: the last three tiles split the cast 50/50 ACT/DVE and
  alternate PSUM-bank copies between ACT and DVE, and the drain tile's
  cast halves both run on ACT -- so no single engine serializes the
  post-stream drain.
- Emission is software-pipelined with a 3-stage skew (cast_m,
  transpose_{m-1}, matmul_{m-2}, epilogue_{m-3}): engine queues are
  strict in-order FIFOs, so without the skew the next tile's cast queues
  behind an epilogue op that waits on the current tile's whole
  cross-engine chain, serializing the pipeline.
- xsum rides the ones column of the fp16 matmul: end-to-end rel err
  ~2.4e-4 against the fp64 reference (gate 2e-2).
- out_stage[p, q] is row 16p+q: single direct store, no final transpose.

Measured on 8 tunneled trn2 cores: ~115-120 us typical, best 114.6
(baseline 141 us); pure-DMA floor of the same schedule ~102 us; run
variance is thermal.
"""

from contextlib import ExitStack

import numpy as np

import concourse.bass as bass
import concourse.mybir as mybir
import concourse.tile as tile
from concourse import bacc
from concourse.bass import ts
from concourse.bass_utils import run_bass_kernel_spmd
from concourse.masks import make_identity

N_CORES = 8
B_FULL = 16384
N_DIM = 4096
K_DIM = 128
B_SHARD = B_FULL // N_CORES  # 2048
G = N_DIM // 128  # 32 contraction chunks
M_TILES = B_SHARD // 128  # 16
F32 = mybir.dt.float32
F16 = mybir.dt.float16
BF16 = mybir.dt.bfloat16
AF = mybir.ActivationFunctionType
ALU = mybir.AluOpType

CHUNKS = (3, 3, 3, 3, 2, 1, 1)  # m-tiles per x DMA; tapered tail for short drain
ACT_COLS = 3072  # cast split: ACT does [0, 3072), DVE does the rest


def build_program(dtype_mode="fp16", nf_pad=132, chunks=CHUNKS, mode="full",
                  act_cols=ACT_COLS, psy_bufs=3, store_split=0, tail_split=1,
                  drain_act=True, tail_bal=2):
    assert sum(chunks) == M_TILES
    cmax = max(chunks)
    mm_dt = {"fp16": F16, "bf16": BF16}[dtype_mode]

    nc = bacc.Bacc("TRN2", target_bir_lowering=False, debug=False)
    x_d = nc.dram_tensor("x", [B_SHARD, N_DIM], F32, kind="ExternalInput").ap()
    # host pre-shuffled: m_d[p, g, n] = M[g*128+p, n]
    m_d = nc.dram_tensor("mw", [128, G, nf_pad], mm_dt, kind="ExternalInput").ap()
    aux_d = nc.dram_tensor("aux", [128, 2], F32, kind="ExternalInput").ap()
    out_d = nc.dram_tensor("out", [B_SHARD, 1], F32, kind="ExternalOutput").ap()

    with tile.TileContext(nc) as tc, ExitStack() as ctx:
        const_pool = ctx.enter_context(tc.tile_pool(name="const", bufs=1))
        x_pool = ctx.enter_context(tc.tile_pool(name="xin", bufs=2))
        xh_pool = ctx.enter_context(tc.tile_pool(name="xh", bufs=3))
        xt_pool = ctx.enter_context(tc.tile_pool(name="xt", bufs=2))
        sc_pool = ctx.enter_context(tc.tile_pool(name="scratch", bufs=2))
        pst_pool = ctx.enter_context(tc.tile_pool(name="pst", bufs=4, space="PSUM"))
        psy_pool = ctx.enter_context(tc.tile_pool(name="psy", bufs=psy_bufs, space="PSUM"))

        # Everything on the one SP HWDGE queue: chunk 0 first (critical
        # path), then the small consts (land by ~34 us, first matmul ~37),
        # then the remaining chunks.  The final tile is loaded as two
        # column-half DMAs so its cast can start before the last byte.
        xq = x_d.rearrange("(p q) n -> p q n", p=128)
        xck = []  # (tile, c, q0)
        q0 = 0
        aux_sb = m_sb = None
        for ci, c in enumerate(chunks):
            xc = x_pool.tile([128, cmax, N_DIM], F32)
            last = ci == len(chunks) - 1
            if last and c == 1:
                h = N_DIM // 2
                nc.sync.dma_start(xc[:, 0:1, 0:h], xq[:, q0 : q0 + 1, 0:h])
                nc.sync.dma_start(xc[:, 0:1, h:], xq[:, q0 : q0 + 1, h:])
            else:
                nc.sync.dma_start(xc[:, 0:c], xq[:, q0 : q0 + c])
            xck.append((xc, c, q0))
            q0 += c
            if ci == 0:
                aux_sb = const_pool.tile([128, 2], F32)
                nc.sync.dma_start(aux_sb[:], aux_d[:])
                m_sb = const_pool.tile([128, G, nf_pad], mm_dt)
                nc.sync.dma_start(m_sb[:], m_d[:])

        ident = const_pool.tile([128, 128], mm_dt)
        make_identity(nc, ident[:])

        out_stage = const_pool.tile([128, M_TILES], F32)

        # Flatten (chunk, slot) -> per-m-tile source slices.
        srcs = []
        for xc, c, q0 in xck:
            for j in range(c):
                srcs.append((xc, j))

        if mode == "dma":
            for m in range(M_TILES):
                xc, j = srcs[m]
                nc.vector.tensor_copy(out_stage[:, m : m + 1], xc[:, j, 0:1])
        else:
            # Software-pipelined emission with a 3-stage skew so no engine's
            # FIFO ever queues an instruction behind one that waits on the
            # current tile's cross-engine chain (ACT: casts never sit behind
            # epilogue squares; DVE: cast-copies never behind combines).
            xhs, xTs, psys, accs = {}, {}, {}, {}

            def st_cast(m):
                xc, j = srcs[m]
                xh = xh_pool.tile([128, N_DIM], mm_dt)
                if m == M_TILES - 1:
                    # Drain tile: both halves on ACT (two slice-dependent
                    # instructions) -- ACT is idle here while DVE still
                    # drains tile-14 copies, so the chain starts at the
                    # half-DMA boundaries instead of behind the DVE FIFO.
                    h = N_DIM // 2
                    nc.scalar.activation(
                        xh[:, 0:h], xc[:, j, 0:h], AF.Identity
                    )
                    if drain_act:
                        nc.scalar.activation(
                            xh[:, h:], xc[:, j, h:], AF.Identity
                        )
                    else:
                        nc.vector.tensor_copy(xh[:, h:], xc[:, j, h:])
                elif m >= M_TILES - 1 - tail_bal:
                    h = N_DIM // 2
                    nc.scalar.activation(
                        xh[:, 0:h], xc[:, j, 0:h], AF.Identity
                    )
                    nc.vector.tensor_copy(xh[:, h:], xc[:, j, h:])
                elif m >= M_TILES - tail_split:
                    h = N_DIM // 2
                    nc.scalar.activation(
                        xh[:, 0:h], xc[:, j, 0:h], AF.Identity
                    )
                    nc.vector.tensor_copy(xh[:, h:], xc[:, j, h:])
                else:
                    nc.scalar.activation(
                        xh[:, 0:act_cols], xc[:, j, 0:act_cols], AF.Identity
                    )
                    nc.vector.tensor_copy(
                        xh[:, act_cols:], xc[:, j, act_cols:]
                    )
                xhs[m] = xh

            def tr_batch(xh, xT, q, bal=False):
                pst = pst_pool.tile([128, 8, 128], mm_dt)
                for jj in range(8):
                    g = 8 * q + jj
                    nc.tensor.transpose(pst[:, jj], xh[:, ts(g, 128)], ident[:])
                if bal and q % 2 == 1:
                    nc.scalar.copy(xT[:, ts(q, 8)], pst[:])
                else:
                    nc.vector.tensor_copy(xT[:, ts(q, 8)], pst[:])

            def st_transpose(m):
                xh = xhs.pop(m)
                xT = xt_pool.tile([128, G, 128], mm_dt)
                bal = m >= M_TILES - 1 - tail_bal
                if m == M_TILES - 1:
                    for q in range(2):
                        tr_batch(xh, xT, q, bal)
                    xhs[m] = xh  # rest in st_matmul
                else:
                    for q in range(G // 8):
                        tr_batch(xh, xT, q, bal)
                xTs[m] = xT

            def st_matmul(m):
                xT = xTs.pop(m)
                psy = psy_pool.tile([128, nf_pad], F32)
                if m == M_TILES - 1:
                    xh = xhs.pop(m)
                    for g in range(G // 2):
                        nc.tensor.matmul(
                            psy[:], lhsT=xT[:, g], rhs=m_sb[:, g],
                            start=(g == 0), stop=False,
                        )
                    for q in range(2, 4):
                        tr_batch(xh, xT, q, m >= M_TILES - 1 - tail_bal)
                    for g in range(G // 2, G):
                        nc.tensor.matmul(
                            psy[:], lhsT=xT[:, g], rhs=m_sb[:, g],
                            start=False, stop=(g == G - 1),
                        )
                else:
                    for g in range(G):
                        nc.tensor.matmul(
                            psy[:], lhsT=xT[:, g], rhs=m_sb[:, g],
                            start=(g == 0), stop=(g == G - 1),
                        )
                psys[m] = psy

            def st_epilogue(m):
                # All-DVE so ACT stays a pure cast engine (no FIFO coupling).
                # Copy psy to SBUF first (frees the PSUM bank, and STT can
                # only take one PSUM operand anyway).
                psy = psys.pop(m)
                ycp = sc_pool.tile([128, K_DIM + 2], F32)
                nc.vector.tensor_copy(ycp[:], psy[:, 0 : K_DIM + 2])
                scr = sc_pool.tile([128, K_DIM], F32)
                sq_acc = sc_pool.tile([128, 1], F32)
                # sq_acc = sum_k 0.5*y_k^2
                nc.vector.scalar_tensor_tensor(
                    out=scr[:], in0=ycp[:, 0:K_DIM], scalar=0.5,
                    in1=ycp[:, 0:K_DIM], op0=ALU.mult, op1=ALU.mult,
                    accum_out=sq_acc[:],
                )
                # t3 = (xsum * 0.5c) * xsum
                t3 = sc_pool.tile([128, 1], F32)
                nc.vector.scalar_tensor_tensor(
                    out=t3[:], in0=ycp[:, K_DIM + 1 : K_DIM + 2],
                    scalar=aux_sb[:, 1:2], in1=ycp[:, K_DIM + 1 : K_DIM + 2],
                    op0=ALU.mult, op1=ALU.mult,
                )
                u = sc_pool.tile([128, 1], F32)
                nc.vector.scalar_tensor_tensor(
                    out=u[:], in0=sq_acc[:], scalar=0.0, in1=t3[:],
                    op0=ALU.add, op1=ALU.subtract,
                )
                nc.vector.scalar_tensor_tensor(
                    out=out_stage[:, m : m + 1],
                    in0=ycp[:, K_DIM : K_DIM + 1],
                    scalar=aux_sb[:, 0:1], in1=u[:], op0=ALU.add, op1=ALU.add,
                )

            for step in range(M_TILES + 3):
                if step < M_TILES:
                    st_cast(step)
                if 1 <= step <= M_TILES:
                    st_transpose(step - 1)
                if 2 <= step <= M_TILES + 1:
                    st_matmul(step - 2)
                if 3 <= step:
                    st_epilogue(step - 3)

        # p-major: out_stage[p, q] is already row 16p+q -> direct store.
        oq = out_d.rearrange("(p q) o -> p q o", p=128)
        if store_split:
            s = store_split
            nc.sync.dma_start(oq[:, 0:s], out_stage[:, 0:s].rearrange("p (q o) -> p q o", o=1))
            nc.sync.dma_start(oq[:, s:], out_stage[:, s:].rearrange("p (q o) -> p q o", o=1))
        else:
            nc.sync.dma_start(
                out_d.rearrange("(p q) o -> p (q o)", p=128), out_stage[:]
            )

    nc.compile()
    return nc


def host_prep(x, W, b, V, nf_pad=132, dtype_mode="fp16"):
    import ml_dtypes

    x = np.ascontiguousarray(x, dtype=np.float32)
    W = np.asarray(W, dtype=np.float32)
    b = np.asarray(b, dtype=np.float32)
    V = np.asarray(V, dtype=np.float32)

    np_dt = {"fp16": np.float16, "bf16": ml_dtypes.bfloat16}[dtype_mode]
    M = np.zeros((N_DIM, nf_pad), dtype=np.float32)
    M[:, :K_DIM] = V
    M[:, K_DIM] = W[0]
    M[:, K_DIM + 1] = 1.0
    # pre-shuffle to [128, G, nf]: partition p holds rows {g*128+p}
    M = np.ascontiguousarray(
        M.reshape(G, 128, nf_pad).transpose(1, 0, 2)
    ).astype(np_dt)

    s = V.astype(np.float64).sum(axis=0)
    c = float(s @ s)
    aux = np.zeros((128, 2), dtype=np.float32)
    aux[:, 0] = b[0]
    aux[:, 1] = 0.5 * c

    return [
        {"x": x[core * B_SHARD : (core + 1) * B_SHARD], "mw": M, "aux": aux}
        for core in range(N_CORES)
    ]


_prog_cache = {}


def _get_program(key):
    if key not in _prog_cache:
        (dtype_mode, nf_pad, chunks, mode, act_cols, psy_bufs, store_split,
         tail_split, drain_act, tail_bal) = key
        _prog_cache[key] = build_program(dtype_mode, nf_pad, chunks, mode,
                                         act_cols, psy_bufs, store_split,
                                         tail_split, drain_act, tail_bal)
    return _prog_cache[key]


def run(x, W, b, V, trace=False, retries=4, dtype_mode="fp16", chunks=CHUNKS,
        mode="full", act_cols=ACT_COLS, psy_bufs=3, store_split=0,
        tail_split=1, drain_act=True, tail_bal=2, **kw):
    nc = _get_program((dtype_mode, 132, tuple(chunks), mode, act_cols,
                       psy_bufs, store_split, tail_split, drain_act,
                       tail_bal))
    in_maps = host_prep(x, W, b, V, nf_pad=132, dtype_mode=dtype_mode)
    last_exc = None
    for attempt in range(retries):
        try:
            res = run_bass_kernel_spmd(nc, in_maps, core_ids=list(range(N_CORES)),
                                       trace=trace, **kw)
            break
        except Exception as e:
            last_exc = e
            import time as _time

            print(f"kernel attempt {attempt} failed ({type(e).__name__}); retrying")
            _time.sleep(2.0)
    else:
        raise last_exc
    out = np.concatenate([r["out"] for r in res.results], axis=0)
    return out, res




# Compatibility aliases for test.py
NF = K_DIM + 2
NF_PAD = 132
DTYPE_MODE = "fp16"

def kernel(x, W, b, V):
    out, _ = run(x, W, b, V)
    return out


# revision 7
# speedup vs baseline: 1.0896x; 1.0896x over previous
"""FM layer (factorization machine) Trainium2 Bass kernel.

Computes, for x (B=16384, N=4096), W (1, N), b (1,), V (N, K=128):
    out = x @ W.T + b + 0.5*sum((x@V)**2, axis=1)
          - 0.5*||V.sum(0)||^2 * (x.sum(1))**2

Data-parallel over B across 8 NeuronCores (2048 rows/core). The kernel is
HBM-bound: reading x (33.5 MB/core fp32) at the ~2.9 TB/s chip HBM ceiling
takes ~93 us; all compute hides under the stream.

Per-core design (p-major rows: SBUF partition p holds rows [16p, 16p+16)):
- x streams HBM->SBUF fp32 via HWDGE on one queue in big tapered chunks
  (6 MB head reads -> one contiguous 64 KB+ run per partition; 2 MB tail
  chunks so late tiles arrive spread out; the final tile in two
  column-half DMAs so its cast starts before the last byte). 2-deep ring;
  small consts ([V|W.T|1] pre-shuffled fp16 + aux) FIFO right after
  chunk 0 on the same queue (a second HWDGE queue gets starved by
  packet-level round-robin against 64 KB descriptors; SWDGE descriptor
  generation is far too slow).
- Per 128-row m-tile: fp32->fp16 cast split ACT cols [0,3072) / DVE rest;
  PE identity-transposes the 32 n-chunks (fp16, ~66 ns each); DVE copies
  each full PSUM bank (8 transposes) back to SBUF; PE matmuls
  y = x_tile @ [V | W.T | 1] accumulating fp32 in PSUM; all-DVE epilogue
  (STT with accum_out gives sum(0.5*y^2) in one op).
- Tail balancing: the last three tiles split the cast 50/50 ACT/DVE and
  alternate PSUM-bank copies between ACT and DVE, and the drain tile's
  cast halves both run on ACT -- so no single engine serializes the
  post-stream drain.
- Emission is software-pipelined with a 3-stage skew (cast_m,
  transpose_{m-1}, matmul_{m-2}, epilogue_{m-3}): engine queues are
  strict in-order FIFOs, so without the skew the next tile's cast queues
  behind an epilogue op that waits on the current tile's whole
  cross-engine chain, serializing the pipeline.
- xsum rides the ones column of the fp16 matmul: end-to-end rel err
  ~2.4e-4 against the fp64 reference (gate 2e-2).
- out_stage[p, q] is row 16p+q: single direct store, no final transpose.

Measured on 8 tunneled trn2 cores: ~115-120 us typical, best 114.6
(baseline 141 us); pure-DMA floor of the same schedule ~102 us; run
variance is thermal.
"""

from contextlib import ExitStack

import numpy as np

import concourse.bass as bass
import concourse.mybir as mybir
import concourse.tile as tile
from concourse import bacc
from concourse.bass import ts
from concourse.bass_utils import run_bass_kernel_spmd
from concourse.masks import make_identity

N_CORES = 8
B_FULL = 16384
N_DIM = 4096
K_DIM = 128
B_SHARD = B_FULL // N_CORES  # 2048
G = N_DIM // 128  # 32 contraction chunks
M_TILES = B_SHARD // 128  # 16
F32 = mybir.dt.float32
F16 = mybir.dt.float16
BF16 = mybir.dt.bfloat16
AF = mybir.ActivationFunctionType
ALU = mybir.AluOpType

CHUNKS = (3, 3, 3, 3, 2, 1, 1)  # m-tiles per x DMA; tapered tail for short drain
ACT_COLS = 3072  # cast split: ACT does [0, 3072), DVE does the rest


def build_program(dtype_mode="fp16", nf_pad=132, chunks=CHUNKS, mode="full",
                  act_cols=ACT_COLS, psy_bufs=3, store_split=0, tail_split=1,
                  drain_act=True, tail_bal=2):
    assert sum(chunks) == M_TILES
    cmax = max(chunks)
    mm_dt = {"fp16": F16, "bf16": BF16}[dtype_mode]

    nc = bacc.Bacc("TRN2", target_bir_lowering=False, debug=False)
    x_d = nc.dram_tensor("x", [B_SHARD, N_DIM], F32, kind="ExternalInput").ap()
    # host pre-shuffled: m_d[p, g, n] = M[g*128+p, n]
    m_d = nc.dram_tensor("mw", [128, G, nf_pad], mm_dt, kind="ExternalInput").ap()
    aux_d = nc.dram_tensor("aux", [128, 2], F32, kind="ExternalInput").ap()
    out_d = nc.dram_tensor("out", [B_SHARD, 1], F32, kind="ExternalOutput").ap()

    with tile.TileContext(nc) as tc, ExitStack() as ctx:
        const_pool = ctx.enter_context(tc.tile_pool(name="const", bufs=1))
        x_pool = ctx.enter_context(tc.tile_pool(name="xin", bufs=2))
        xh_pool = ctx.enter_context(tc.tile_pool(name="xh", bufs=3))
        xt_pool = ctx.enter_context(tc.tile_pool(name="xt", bufs=2))
        sc_pool = ctx.enter_context(tc.tile_pool(name="scratch", bufs=2))
        pst_pool = ctx.enter_context(tc.tile_pool(name="pst", bufs=4, space="PSUM"))
        psy_pool = ctx.enter_context(tc.tile_pool(name="psy", bufs=psy_bufs, space="PSUM"))

        # Everything on the one SP HWDGE queue: chunk 0 first (critical
        # path), then the small consts (land by ~34 us, first matmul ~37),
        # then the remaining chunks.  The final tile is loaded as two
        # column-half DMAs so its cast can start before the last byte.
        xq = x_d.rearrange("(p q) n -> p q n", p=128)
        xck = []  # (tile, c, q0)
        q0 = 0
        aux_sb = m_sb = None
        for ci, c in enumerate(chunks):
            xc = x_pool.tile([128, cmax, N_DIM], F32)
            last = ci == len(chunks) - 1
            if last and c == 1:
                h = N_DIM // 2
                nc.sync.dma_start(xc[:, 0:1, 0:h], xq[:, q0 : q0 + 1, 0:h])
                nc.sync.dma_start(xc[:, 0:1, h:], xq[:, q0 : q0 + 1, h:])
            else:
                nc.sync.dma_start(xc[:, 0:c], xq[:, q0 : q0 + c])
            xck.append((xc, c, q0))
            q0 += c
            if ci == 0:
                aux_sb = const_pool.tile([128, 2], F32)
                nc.sync.dma_start(aux_sb[:], aux_d[:])
                m_sb = const_pool.tile([128, G, nf_pad], mm_dt)
                nc.sync.dma_start(m_sb[:], m_d[:])

        ident = const_pool.tile([128, 128], mm_dt)
        make_identity(nc, ident[:])

        out_stage = const_pool.tile([128, M_TILES], F32)

        # Flatten (chunk, slot) -> per-m-tile source slices.
        srcs = []
        for xc, c, q0 in xck:
            for j in range(c):
                srcs.append((xc, j))

        if mode == "dma":
            for m in range(M_TILES):
                xc, j = srcs[m]
                nc.vector.tensor_copy(out_stage[:, m : m + 1], xc[:, j, 0:1])
        else:
            # Software-pipelined emission with a 3-stage skew so no engine's
            # FIFO ever queues an instruction behind one that waits on the
            # current tile's cross-engine chain (ACT: casts never sit behind
            # epilogue squares; DVE: cast-copies never behind combines).
            xhs, xTs, psys, accs = {}, {}, {}, {}

            def st_cast(m):
                xc, j = srcs[m]
                xh = xh_pool.tile([128, N_DIM], mm_dt)
                if m == M_TILES - 1:
                    # Drain tile: both halves on ACT (two slice-dependent
                    # instructions) -- ACT is idle here while DVE still
                    # drains tile-14 copies, so the chain starts at the
                    # half-DMA boundaries instead of behind the DVE FIFO.
                    h = N_DIM // 2
                    nc.scalar.activation(
                        xh[:, 0:h], xc[:, j, 0:h], AF.Identity
                    )
                    if drain_act:
                        nc.scalar.activation(
                            xh[:, h:], xc[:, j, h:], AF.Identity
                        )
                    else:
                        nc.vector.tensor_copy(xh[:, h:], xc[:, j, h:])
                elif m >= M_TILES - 1 - tail_bal:
                    h = N_DIM // 2
                    nc.scalar.activation(
                        xh[:, 0:h], xc[:, j, 0:h], AF.Identity
                    )
                    nc.vector.tensor_copy(xh[:, h:], xc[:, j, h:])
                elif m >= M_TILES - tail_split:
                    h = N_DIM // 2
                    nc.scalar.activation(
                        xh[:, 0:h], xc[:, j, 0:h], AF.Identity
                    )
                    nc.vector.tensor_copy(xh[:, h:], xc[:, j, h:])
                else:
                    nc.scalar.activation(
                        xh[:, 0:act_cols], xc[:, j, 0:act_cols], AF.Identity
                    )
                    nc.vector.tensor_copy(
                        xh[:, act_cols:], xc[:, j, act_cols:]
                    )
                xhs[m] = xh

            def tr_batch(xh, xT, q, bal=False):
                pst = pst_pool.tile([128, 8, 128], mm_dt)
                for jj in range(8):
                    g = 8 * q + jj
                    nc.tensor.transpose(pst[:, jj], xh[:, ts(g, 128)], ident[:])
                if bal and q % 2 == 1:
                    nc.scalar.copy(xT[:, ts(q, 8)], pst[:])
                else:
                    nc.vector.tensor_copy(xT[:, ts(q, 8)], pst[:])

            def st_transpose(m):
                xh = xhs.pop(m)
                xT = xt_pool.tile([128, G, 128], mm_dt)
                bal = m >= M_TILES - 1 - tail_bal
                if m == M_TILES - 1:
                    for q in range(2):
                        tr_batch(xh, xT, q, bal)
                    xhs[m] = xh  # rest in st_matmul
                else:
                    for q in range(G // 8):
                        tr_batch(xh, xT, q, bal)
                xTs[m] = xT

            def st_matmul(m):
                xT = xTs.pop(m)
                psy = psy_pool.tile([128, nf_pad], F32)
                if m == M_TILES - 1:
                    xh = xhs.pop(m)
                    for g in range(G // 2):
                        nc.tensor.matmul(
                            psy[:], lhsT=xT[:, g], rhs=m_sb[:, g],
                            start=(g == 0), stop=False,
                        )
                    for q in range(2, 4):
                        tr_batch(xh, xT, q, m >= M_TILES - 1 - tail_bal)
                    for g in range(G // 2, G):
                        nc.tensor.matmul(
                            psy[:], lhsT=xT[:, g], rhs=m_sb[:, g],
                            start=False, stop=(g == G - 1),
                        )
                else:
                    for g in range(G):
                        nc.tensor.matmul(
                            psy[:], lhsT=xT[:, g], rhs=m_sb[:, g],
                            start=(g == 0), stop=(g == G - 1),
                        )
                psys[m] = psy

            def st_epilogue(m):
                # All-DVE so ACT stays a pure cast engine (no FIFO coupling).
                # Copy psy to SBUF first (frees the PSUM bank, and STT can
                # only take one PSUM operand anyway).
                psy = psys.pop(m)
                ycp = sc_pool.tile([128, K_DIM + 2], F32)
                nc.vector.tensor_copy(ycp[:], psy[:, 0 : K_DIM + 2])
                scr = sc_pool.tile([128, K_DIM], F32)
                sq_acc = sc_pool.tile([128, 1], F32)
                # sq_acc = sum_k 0.5*y_k^2
                nc.vector.scalar_tensor_tensor(
                    out=scr[:], in0=ycp[:, 0:K_DIM], scalar=0.5,
                    in1=ycp[:, 0:K_DIM], op0=ALU.mult, op1=ALU.mult,
                    accum_out=sq_acc[:],
                )
                # t3 = (xsum * 0.5c) * xsum
                t3 = sc_pool.tile([128, 1], F32)
                nc.vector.scalar_tensor_tensor(
                    out=t3[:], in0=ycp[:, K_DIM + 1 : K_DIM + 2],
                    scalar=aux_sb[:, 1:2], in1=ycp[:, K_DIM + 1 : K_DIM + 2],
                    op0=ALU.mult, op1=ALU.mult,
                )
                u = sc_pool.tile([128, 1], F32)
                nc.vector.scalar_tensor_tensor(
                    out=u[:], in0=sq_acc[:], scalar=0.0, in1=t3[:],
                    op0=ALU.add, op1=ALU.subtract,
                )
                nc.vector.scalar_tensor_tensor(
                    out=out_stage[:, m : m + 1],
                    in0=ycp[:, K_DIM : K_DIM + 1],
                    scalar=aux_sb[:, 0:1], in1=u[:], op0=ALU.add, op1=ALU.add,
                )

            for step in range(M_TILES + 3):
                if step < M_TILES:
                    st_cast(step)
                if 1 <= step <= M_TILES:
                    st_transpose(step - 1)
                if 2 <= step <= M_TILES + 1:
                    st_matmul(step - 2)
                if 3 <= step:
                    st_epilogue(step - 3)

        # p-major: out_stage[p, q] is already row 16p+q -> direct store.
        oq = out_d.rearrange("(p q) o -> p q o", p=128)
        if store_split:
            s = store_split
            nc.sync.dma_start(oq[:, 0:s], out_stage[:, 0:s].rearrange("p (q o) -> p q o", o=1))
            nc.sync.dma_start(oq[:, s:], out_stage[:, s:].rearrange("p (q o) -> p q o", o=1))
        else:
            nc.sync.dma_start(
                out_d.rearrange("(p q) o -> p (q o)", p=128), out_stage[:]
            )

    nc.compile()
    return nc


def host_prep(x, W, b, V, nf_pad=132, dtype_mode="fp16"):
    import ml_dtypes

    x = np.ascontiguousarray(x, dtype=np.float32)
    W = np.asarray(W, dtype=np.float32)
    b = np.asarray(b, dtype=np.float32)
    V = np.asarray(V, dtype=np.float32)

    np_dt = {"fp16": np.float16, "bf16": ml_dtypes.bfloat16}[dtype_mode]
    M = np.zeros((N_DIM, nf_pad), dtype=np.float32)
    M[:, :K_DIM] = V
    M[:, K_DIM] = W[0]
    M[:, K_DIM + 1] = 1.0
    # pre-shuffle to [128, G, nf]: partition p holds rows {g*128+p}
    M = np.ascontiguousarray(
        M.reshape(G, 128, nf_pad).transpose(1, 0, 2)
    ).astype(np_dt)

    s = V.astype(np.float64).sum(axis=0)
    c = float(s @ s)
    aux = np.zeros((128, 2), dtype=np.float32)
    aux[:, 0] = b[0]
    aux[:, 1] = 0.5 * c

    return [
        {"x": x[core * B_SHARD : (core + 1) * B_SHARD], "mw": M, "aux": aux}
        for core in range(N_CORES)
    ]


_prog_cache = {}


def _get_program(key):
    if key not in _prog_cache:
        (dtype_mode, nf_pad, chunks, mode, act_cols, psy_bufs, store_split,
         tail_split, drain_act, tail_bal) = key
        _prog_cache[key] = build_program(dtype_mode, nf_pad, chunks, mode,
                                         act_cols, psy_bufs, store_split,
                                         tail_split, drain_act, tail_bal)
    return _prog_cache[key]


def run(x, W, b, V, trace=False, retries=4, dtype_mode="fp16", chunks=CHUNKS,
        mode="full", act_cols=ACT_COLS, psy_bufs=3, store_split=0,
        tail_split=1, drain_act=True, tail_bal=2, **kw):
    nc = _get_program((dtype_mode, 132, tuple(chunks), mode, act_cols,
                       psy_bufs, store_split, tail_split, drain_act,
                       tail_bal))
    in_maps = host_prep(x, W, b, V, nf_pad=132, dtype_mode=dtype_mode)
    last_exc = None
    for attempt in range(retries):
        try:
            res = run_bass_kernel_spmd(nc, in_maps, core_ids=list(range(N_CORES)),
                                       trace=trace, **kw)
            break
        except Exception as e:
            last_exc = e
            import time as _time

            print(f"kernel attempt {attempt} failed ({type(e).__name__}); retrying")
            _time.sleep(2.0)
    else:
        raise last_exc
    out = np.concatenate([r["out"] for r in res.results], axis=0)
    return out, res




# Compatibility aliases for test.py
NF = K_DIM + 2
NF_PAD = 132
DTYPE_MODE = "fp16"

def kernel(x, W, b, V):
    out, _ = run(x, W, b, V)
    return out
